# revision 4
# baseline (speedup 1.0000x reference)
"""DYConv2d on 8 trn2 cores: F(2,3)-1D horizontal Winograd in fp16.

Per core: 4 samples. The 3x3 conv is computed as a 1D Winograd F(2,3)
along W (2.25/1.5 = 1.5x fewer PE MACs than direct) with the vertical
3 taps kept direct (accumulated in PSUM):
  per (nu in 0..3, kh in 0..2, ct in 0..1): M_nu += U_nu(kh,ct)^T V_nu
  U0 = w0, U1 = (w0+w1+w2)/2, U2 = (w0-w1+w2)/2, U3 = w2   (per kh)
  V0 = d0-d2, V1 = d1+d2, V2 = d2-d1, V3 = d1-d3            (per col pair)
  out_even = M0+M1+M2, out_odd = M1-M2-M3
Host ships the padded image phase-split ([58, 2, 29]: even/odd columns)
so every DVE transform op is stride-1. Whole datapath fp16 (Winograd in
bf16 fails the 2e-2 tolerance; fp16 sims at ~3e-3).

Attention path (sums -> shared MLP -> 3 sigmoid branches -> colsc outer
product -> per-sample weight synth) is the baseline's, in fp16.
"""

import numpy as np

B, C, O, KS, H, W, R = 32, 256, 256, 3, 56, 56, 16
KK = KS * KS
NCORES = 8
BL = B // NCORES
HP = H + 2            # 58 rows (vertical pad)
WP = W + 2            # 58 padded cols = 2 phases x 29
NPIX = HP * WP        # 3364
NTT = 28              # F(2,3) tiles along W
NU = 4
RGH = 14              # output rows per group
NRG = 4
FREE = RGH * NTT      # 392 matmul free dim

TRACE = False
LAST_EXEC_NS = None
LAST_RESULTS = None

_CACHED = None


def _build_program():
    global _CACHED
    if _CACHED is not None:
        return _CACHED

    from contextlib import ExitStack

    from concourse import bacc
    import concourse.mybir as mybir
    import concourse.tile as tile

    f32 = mybir.dt.float32
    f16 = mybir.dt.float16
    AF = mybir.ActivationFunctionType
    AX = mybir.AxisListType

    nc = bacc.Bacc("TRN2", target_bir_lowering=False, debug=False)

    x_d = nc.dram_tensor("x", [BL, C, HP, WP], f16, kind="ExternalInput").ap()
    bwT_d = nc.dram_tensor("bwT", [2, 128, KK * O], f16, kind="ExternalInput").ap()
    fcsh_d = nc.dram_tensor("fcsh", [2, 128, R], f16, kind="ExternalInput").ap()
    bsh_d = nc.dram_tensor("bsh", [R, 1], f32, kind="ExternalInput").ap()
    fcik_d = nc.dram_tensor("fcik", [R + 1, C + KK + 1], f16, kind="ExternalInput").ap()
    hones_d = nc.dram_tensor("hones", [R + 1, 2], f16, kind="ExternalInput").ap()
    fcoupT_d = nc.dram_tensor("fcoupT", [R + 1, O], f16, kind="ExternalInput").ap()
    out_d = nc.dram_tensor("out", [BL, O, H, W], f32, kind="ExternalOutput").ap()

    with tile.TileContext(nc) as tc, ExitStack() as ctx:
        persist = ctx.enter_context(tc.tile_pool(name="persist", bufs=1))
        conv_psum = ctx.enter_context(
            tc.tile_pool(name="conv_psum", bufs=7, space="PSUM")
        )
        attn_psum = ctx.enter_context(
            tc.tile_pool(name="attn_psum", bufs=1, space="PSUM")
        )
        out_pool = ctx.enter_context(tc.tile_pool(name="out_pool", bufs=6))
        scr_pool = ctx.enter_context(tc.tile_pool(name="scr_pool", bufs=6))

        # warm-up: junk matmuls keep the PE pstate up while the first image
        # streams; junk activations preload ACT tables
        warm = persist.tile([128, 128 + FREE], f16, tag="warm", name="warm")
        nc.gpsimd.memset(warm[:], 0.5)
        warm_act = persist.tile([1, 2], f32, tag="warm_act", name="warm_act")
        nc.scalar.activation(warm_act[:, 0:1], warm[:1, 0:1], AF.Relu)
        nc.scalar.activation(warm_act[:, 1:2], warm[:1, 0:1], AF.Sigmoid)
        warm_ps = conv_psum.tile([128, FREE], f32, tag="cpsum", name="warm_ps")

        def emit_warm_mms(n, w=FREE):
            for _ in range(n):
                nc.tensor.matmul(
                    warm_ps[:, 0:w], warm[:, 0:128], warm[:, 128:128 + w],
                    start=True, stop=True,
                )

        # ---- double-buffered per-sample image (phase-split) ----
        ximg, ximg_v = [], []
        for s in range(2):
            ximg.append([persist.tile([128, NPIX], f16, tag=f"ximg{s}{ct}",
                                      name=f"ximg{s}{ct}") for ct in range(2)])
            ximg_v.append([t[:].rearrange("p (h ph t) -> p h ph t", h=HP, ph=2)
                           for t in ximg[s]])

        HH = NPIX // 2  # 1682
        C1CHUNKS = ((0, HH), (HH, NPIX))

        def emit_img_dma(s, b, split=False):
            # images ride the scalar (ct0) and gpsimd (ct1) rings so they are
            # never queued behind the output stream on the sync ring
            xv0 = x_d[b, 0:128].rearrange("p r c -> p (r c)")
            xv1 = x_d[b, 128:256].rearrange("p r c -> p (r c)")
            insts = [nc.scalar.dma_start(ximg[s][0][:], xv0)]
            for lo, hi in C1CHUNKS:
                insts.append(nc.gpsimd.dma_start(ximg[s][1][:, lo:hi], xv1[:, lo:hi]))
            return insts

        red_scr = persist.tile([128, HH], f16, tag="red_scr", name="red_scr")

        img0_dmas = emit_img_dma(0, 0, split=True)
        h_ext = []
        for s in range(2):
            h_ext.append(persist.tile([R + 1, 2], f16, tag=f"hext{s}", name=f"hext{s}"))
            nc.sync.dma_start(h_ext[s][:], hones_d[:])

        from concourse.tile import add_dep_helper

        fcsh_sb = []
        for ct in range(2):
            t = persist.tile([128, R], f16, tag=f"fcsh{ct}", name=f"fcsh{ct}")
            nc.sync.dma_start(t[:], fcsh_d[ct])
            fcsh_sb.append(t)
        bsh_sb = persist.tile([R, 1], f32, tag="bsh", name="bsh_sb")
        nc.sync.dma_start(bsh_sb[:], bsh_d[:])
        fcik_sb = persist.tile([R + 1, C + KK + 1], f16, tag="fcik", name="fcik_sb")
        nc.sync.dma_start(fcik_sb[:], fcik_d[:])
        fcoupT_sb = persist.tile([R + 1, O], f16, tag="fcoupT", name="fcoupT_sb")
        nc.sync.dma_start(fcoupT_sb[:], fcoupT_d[:])
        bwT_sb = []
        for ct in range(2):
            t = persist.tile([128, KK * O], f16, tag=f"bwT{ct}", name=f"bwT{ct}")
            di = nc.sync.dma_start(t[:], bwT_d[ct])
            add_dep_helper(di.ins, img0_dmas[-1].ins,
                           reason="bwT transfers after sample-0 image")
            bwT_sb.append(t)

        # ---- double-buffered per-sample state ----
        w_sb, ut_sb, V_sb = [], [], []
        s_col, s_part, colsc_sb, ainp_row, aoup_sb = [], [], [], [], []
        for s in range(2):
            w_sb.append([persist.tile([128, KK * O], f16, tag=f"wsb{s}{ct}",
                                      name=f"wsb{s}{ct}") for ct in range(2)])
            ut_sb.append([persist.tile([128, KS, 2, O], f16, tag=f"utsb{s}{ct}",
                                       name=f"utsb{s}{ct}") for ct in range(2)])
            V_sb.append([persist.tile([128, NU, HP, NTT], f16, tag=f"vsb{s}{ct}",
                                      name=f"vsb{s}{ct}") for ct in range(2)])
            colsc_sb.append(persist.tile([128, 2 * KK], f32, tag=f"colsc{s}",
                                         name=f"colsc{s}"))
            s_col.append([persist.tile([128, 2], f16, tag=f"scol{s}{ct}",
                                       name=f"scol{s}{ct}") for ct in range(2)])
            s_part.append([persist.tile([128, 4], f32, tag=f"spart{s}{ct}",
                                        name=f"spart{s}{ct}") for ct in range(2)])
            ainp_row.append(persist.tile([1, C + KK + 1], f16, tag=f"ainp{s}",
                                         name=f"ainp{s}"))
            aoup_sb.append(persist.tile([128, 2], f32, tag=f"aoup{s}",
                                        name=f"aoup{s}"))

        def emit_stage_b_red(s, b):
            iv0 = ximg[s][0][:]
            for hh in range(2):
                nc.vector.reduce_sum(
                    s_part[s][0][:, hh:hh + 1], iv0[:, hh * HH:(hh + 1) * HH],
                    axis=AX.X,
                )
            nc.vector.tensor_add(
                s_col[s][0][:],
                s_part[s][0][:, 0:1].broadcast_to((128, 2)),
                s_part[s][0][:, 1:2].broadcast_to((128, 2)),
            )
            iv1 = ximg[s][1][:]
            for i, (lo, hi) in enumerate(C1CHUNKS):
                nc.scalar.activation(
                    red_scr[:, 0:hi - lo], iv1[:, lo:hi], AF.Copy,
                    accum_out=s_part[s][1][:, i:i + 1],
                )
            nc.vector.tensor_add(
                s_col[s][1][:],
                s_part[s][1][:, 0:1].broadcast_to((128, 2)),
                s_part[s][1][:, 1:2].broadcast_to((128, 2)),
            )

        def emit_stage_b_mm(s, b):
            hp = attn_psum.tile([R, 2], f32, tag="apsum", name="hp")
            nc.tensor.matmul(hp[:], fcsh_sb[0][:], s_col[s][0][:], start=True, stop=False)
            nc.tensor.matmul(hp[:], fcsh_sb[1][:], s_col[s][1][:], start=False, stop=True)
            nc.scalar.activation(h_ext[s][0:R, :], hp[:], AF.Relu, bias=bsh_sb[:])

        def emit_stage_c(s, b):
            ainp_p = attn_psum.tile([2, C + KK + 1], f32, tag="apsum", name="ainp_p")
            nc.tensor.matmul(ainp_p[:], h_ext[s][:], fcik_sb[:], start=True, stop=True)
            nc.scalar.activation(ainp_row[s][:], ainp_p[0:1, :], AF.Sigmoid)

        def emit_stage_d(s, b):
            cs_p = attn_psum.tile([128, 2 * (KK + 1)], f32, tag="apsum", name="cs_p")
            for ct in range(2):
                nc.tensor.matmul(
                    cs_p[:, ct * (KK + 1):(ct + 1) * (KK + 1)],
                    ainp_row[s][:, ct * 128:(ct + 1) * 128],
                    ainp_row[s][:, C:C + KK + 1],
                    start=True, stop=True,
                )
            nc.vector.tensor_copy(
                colsc_sb[s][:].rearrange("p (c k) -> p c k", c=2),
                cs_p[:].rearrange("p (c k) -> p c k", c=2)[:, :, 0:KK],
            )

        def emit_stage_e(s, b, ct):
            # per-(c,k) scalar via tensor_scalar AP column: packed fast mode
            wv = w_sb[s][ct][:].rearrange("p (k o) -> p k o", k=KK)
            bv = bwT_sb[ct][:].rearrange("p (k o) -> p k o", k=KK)
            ALU = mybir.AluOpType
            for k in range(KK):
                nc.vector.tensor_scalar(
                    wv[:, k, :], bv[:, k, :],
                    colsc_sb[s][:, ct * KK + k: ct * KK + k + 1], None,
                    op0=ALU.mult,
                )

        def emit_stage_u(s, b, ct):
            # U1 = (w0+w1+w2)/2, U2 = (w0-w1+w2)/2 per kh (U0/U3 alias w0/w2)
            wv = w_sb[s][ct][:].rearrange("p (kh kw o) -> p kh kw o", kh=KS, kw=KS)
            for kh in range(KS):
                t = scr_pool.tile([128, O], f16, tag="uscr_t", name="uscr_t")
                t2 = scr_pool.tile([128, O], f16, tag="uscr_2", name="uscr_2")
                t3 = scr_pool.tile([128, O], f16, tag="uscr_3", name="uscr_3")
                ALU = mybir.AluOpType
                nc.vector.tensor_add(t[:], wv[:, kh, 0, :], wv[:, kh, 2, :])
                nc.vector.tensor_add(t2[:], t[:], wv[:, kh, 1, :])
                nc.vector.tensor_sub(t3[:], t[:], wv[:, kh, 1, :])
                nc.vector.tensor_scalar(ut_sb[s][ct][:, kh, 0, :], t2[:], 0.5,
                                        None, op0=ALU.mult)
                nc.vector.tensor_scalar(ut_sb[s][ct][:, kh, 1, :], t3[:], 0.5,
                                        None, op0=ALU.mult)

        def emit_stage_v(s, b, ct, nus=(0, 1, 2, 3)):
            # V0 = d0-d2, V1 = d1+d2, V2 = d2-d1, V3 = d1-d3 (stride-1 phases)
            v = ximg_v[s][ct]
            d0 = v[:, :, 0, 0:NTT]
            d1 = v[:, :, 1, 0:NTT]
            d2 = v[:, :, 0, 1:NTT + 1]
            d3 = v[:, :, 1, 1:NTT + 1]
            V = V_sb[s][ct][:]
            if 0 in nus:
                nc.vector.tensor_sub(V[:, 0, :, :], d0, d2)
            if 1 in nus:
                nc.vector.tensor_add(V[:, 1, :, :], d1, d2)
            if 2 in nus:
                nc.vector.tensor_sub(V[:, 2, :, :], d2, d1)
            if 3 in nus:
                nc.vector.tensor_sub(V[:, 3, :, :], d1, d3)

        def emit_stage_f(s, b):
            ao_p = attn_psum.tile([128, 4], f32, tag="apsum", name="ao_p")
            for ot in range(2):
                nc.tensor.matmul(
                    ao_p[:, 2 * ot:2 * ot + 2],
                    fcoupT_sb[:, ot * 128:(ot + 1) * 128],
                    h_ext[s][:],
                    start=True, stop=True,
                )
            nc.scalar.activation(
                aoup_sb[s][:, 0:2],
                ao_p[:].rearrange("p (o c) -> p o c", o=2)[:, :, 0],
                AF.Sigmoid,
            )

        def stat_slice(s, ct, nu, kh, ot):
            if nu == 0:
                k = kh * KS + 0
                return w_sb[s][ct][:, k * O + ot * 128: k * O + ot * 128 + 128]
            if nu == 3:
                k = kh * KS + 2
                return w_sb[s][ct][:, k * O + ot * 128: k * O + ot * 128 + 128]
            return ut_sb[s][ct][:, kh, nu - 1, ot * 128: ot * 128 + 128]

        def emit_conv_rg(s, b, ot, rg, tail=False):
            ps = [conv_psum.tile([128, FREE], f32, tag="cpsum", name=f"cps{nu}")
                  for nu in range(NU)]
            for nu in range(NU):
                t = 0
                for ct in range(2):
                    for kh in range(KS):
                        nc.tensor.matmul(
                            ps[nu][:],
                            stat_slice(s, ct, nu, kh, ot),
                            V_sb[s][ct][
                                :, nu, rg * RGH + kh: rg * RGH + kh + RGH, :],
                            start=(t == 0), stop=(t == 5),
                        )
                        t += 1
            # inverse transform + evac: out_even = M0+M1+M2, out_odd = M1-M2-M3.
            # HW limit: one PSUM operand per tensor_tensor. ACT stages M1, M2
            # so two of the four DVE ops run SBUF-only (packed fast mode).
            m1s = scr_pool.tile([128, FREE], f16, tag="inv_m1", name="inv_m1")
            m2s = scr_pool.tile([128, FREE], f16, tag="inv_m2", name="inv_m2")
            tsc = scr_pool.tile([128, FREE], f16, tag="inv_t", name="inv_t")
            usc = scr_pool.tile([128, FREE], f16, tag="inv_u", name="inv_u")
            ostg = out_pool.tile([128, RGH * W], f16, tag="ostg", name="ostg")
            ow = ostg[:].rearrange("p (h t two) -> p h t two", h=RGH, two=2)
            nc.scalar.activation(m1s[:], ps[1][:], AF.Copy)
            nc.scalar.activation(m2s[:], ps[2][:], AF.Copy)
            nc.vector.tensor_add(tsc[:], m1s[:], ps[0][:])
            nc.vector.tensor_sub(usc[:], m1s[:], m2s[:])
            tv = tsc[:].rearrange("p (h t) -> p h t", h=RGH)
            uv = usc[:].rearrange("p (h t) -> p h t", h=RGH)
            m2v = m2s[:].rearrange("p (h t) -> p h t", h=RGH)
            p3v = ps[3][:].rearrange("p (h t) -> p h t", h=RGH)
            nc.vector.tensor_add(ow[:, :, :, 0], tv, m2v)
            nc.vector.tensor_sub(ow[:, :, :, 1], uv, p3v)
            osb = out_pool.tile([128, RGH * W], f32, tag="osb", name="osb")
            od = out_d[b, ot * 128:(ot + 1) * 128, rg * RGH:(rg + 1) * RGH, :]
            ov = osb[:].rearrange("p (r c) -> p r c", r=RGH)
            if tail:
                # final row group: halve + parallelize so the kernel tail only
                # waits on a 7-row transfer per ring
                HF = RGH // 2
                for half, eng in ((0, nc.sync), (1, nc.scalar)):
                    nc.scalar.activation(
                        osb[:, half * HF * W:(half + 1) * HF * W],
                        ostg[:, half * HF * W:(half + 1) * HF * W],
                        AF.Copy, scale=aoup_sb[s][:, ot:ot + 1])
                    eng.dma_start(od[:, half * HF:(half + 1) * HF, :],
                                  ov[:, half * HF:(half + 1) * HF, :])
            else:
                nc.scalar.activation(osb[:], ostg[:], AF.Copy,
                                     scale=aoup_sb[s][:, ot:ot + 1])
                nc.sync.dma_start(od, ov)

        # ---- head: sample 0's attention + transform chain under junk mms ----
        emit_stage_b_red(0, 0)
        emit_warm_mms(30)
        emit_stage_b_mm(0, 0)
        emit_warm_mms(4, w=128)
        emit_stage_c(0, 0)
        emit_warm_mms(5, w=128)
        emit_stage_d(0, 0)
        emit_stage_f(0, 0)
        emit_warm_mms(8, w=128)
        emit_stage_e(0, 0, 0)
        emit_stage_e(0, 0, 1)
        emit_stage_u(0, 0, 0)
        emit_stage_u(0, 0, 1)
        emit_warm_mms(4, w=128)
        emit_stage_v(0, 0, 0)
        emit_stage_v(0, 0, 1)

        for b in range(BL):
            s = b % 2
            sn = (b + 1) % 2
            if b + 1 < BL:
                # producers placed well before their PE consumers; V transform
                # (the big DVE burst) spread over three boundaries so the next
                # sample's first GEMM never waits on it
                interleave = {
                    0: (lambda sn=sn, b=b: emit_img_dma(sn, b + 1, split=True)),
                    2: (lambda sn=sn, b=b: emit_stage_b_red(sn, b + 1)),
                    3: (lambda sn=sn, b=b: emit_stage_b_mm(sn, b + 1)),
                    4: (lambda sn=sn, b=b: (emit_stage_c(sn, b + 1),
                                            emit_stage_d(sn, b + 1))),
                    5: (lambda sn=sn, b=b: (emit_stage_f(sn, b + 1),
                                            emit_stage_e(sn, b + 1, 0),
                                            emit_stage_e(sn, b + 1, 1),
                                            emit_stage_v(sn, b + 1, 0, (0, 1)))),
                    6: (lambda sn=sn, b=b: (emit_stage_u(sn, b + 1, 0),
                                            emit_stage_u(sn, b + 1, 1),
                                            emit_stage_v(sn, b + 1, 0, (2, 3)),
                                            emit_stage_v(sn, b + 1, 1, (0, 1)))),
                    7: (lambda sn=sn, b=b: emit_stage_v(sn, b + 1, 1, (2, 3))),
                }
            else:
                interleave = {}
            blk = 0
            for ot in range(2):
                for rg in range(NRG):
                    emit_conv_rg(s, b, ot, rg,
                                 tail=(b == BL - 1 and ot == 1 and rg == NRG - 1))
                    fn = interleave.pop(blk, None)
                    if fn is not None:
                        fn()
                    blk += 1

    nc.compile()
    _CACHED = nc
    return nc


def kernel(x, base_w, fc_share_w, fc_share_b, fc_inp_w, fc_inp_b,
           fc_oup_w, fc_oup_b, fc_k_w, fc_k_b):
    global LAST_EXEC_NS, LAST_RESULTS
    from concourse.bass_utils import run_bass_kernel_spmd

    fp16 = np.float16
    nc = _build_program()

    x = np.asarray(x, np.float32)
    # host: zero-pad then phase-split columns (even/odd) so the device
    # Winograd column ops are all stride-1
    xp = np.zeros((B, C, HP, WP), np.float32)
    xp[:, :, 1:H + 1, 1:W + 1] = x
    xph = np.ascontiguousarray(
        xp.reshape(B, C, HP, WP // 2, 2).transpose(0, 1, 2, 4, 3)
    ).reshape(B, C, HP, WP).astype(fp16)

    bwT = np.ascontiguousarray(
        np.asarray(base_w, np.float32).transpose(1, 2, 3, 0).reshape(2, 128, KK * O)
    ).astype(fp16)
    fcsh = np.ascontiguousarray(
        (np.asarray(fc_share_w, np.float32) / float(H * W)).T.reshape(2, 128, R)
    ).astype(fp16)
    bsh = np.ascontiguousarray(np.asarray(fc_share_b, np.float32).reshape(R, 1))
    fcinT = np.concatenate([np.asarray(fc_inp_w, np.float32).T,
                            np.asarray(fc_inp_b, np.float32)[None, :]], axis=0)
    fckT = np.concatenate([np.asarray(fc_k_w, np.float32).T,
                           np.asarray(fc_k_b, np.float32)[None, :]], axis=0)
    fcik = np.ascontiguousarray(
        np.concatenate([fcinT, fckT, np.zeros((R + 1, 1), np.float32)], axis=1)
    ).astype(fp16)
    hones = np.ones((R + 1, 2), fp16)
    fcoupT = np.ascontiguousarray(
        np.concatenate([np.asarray(fc_oup_w, np.float32).T,
                        np.asarray(fc_oup_b, np.float32)[None, :]], axis=0)
    ).astype(fp16)

    in_maps = []
    for i in range(NCORES):
        in_maps.append(
            {
                "x": np.ascontiguousarray(xph[i * BL:(i + 1) * BL]),
                "bwT": bwT,
                "fcsh": fcsh,
                "bsh": bsh,
                "fcik": fcik,
                "hones": hones,
                "fcoupT": fcoupT,
            }
        )

    res = run_bass_kernel_spmd(nc, in_maps, list(range(NCORES)), trace=TRACE)
    LAST_EXEC_NS = res.exec_time_ns
    LAST_RESULTS = res
    return np.concatenate([res.results[i]["out"] for i in range(NCORES)], axis=0)
